# revision 8
# baseline (speedup 1.0000x reference)
"""Mixture-of-Experts (B=4, S=2048, D=1024, F=4096, E=8, top-2) on 8 trn2 NeuronCores.

Strategy: expert parallelism, one expert per core.
  - Host: gate (softmax + top-2 + renorm) in float64, dispatch (gather) tokens
    per expert, pad to a common capacity C, pack all device tensors so every
    DMA moves 8-16KB contiguous per SBUF partition (big-packet DMA).
  - Device (SPMD, identical program, per-core data): y^T = W2^T @ gelu(W1^T @ x^T + b1) + b2
    with both weights resident in SBUF as bf16 and tokens streamed in chunks
    of 512. PSUM accumulates over the contraction (D resp. F) in fp32.
    ~100 tiny warmup matmuls run during the initial weight DMA so the PE HAM
    clock-gate is already at 8/8 when the real matmuls start.
  - Host: combine with the gate weights (y *= cw) and scatter-add back into
    the [B*S, D] output. Token index sets are unique per expert, so fancy-index
    add per expert is race-free.
"""

import copy
import sys

import numpy as np

for _p in ("/opt/trn_rl_repo", "/opt/pypackages"):
    if _p not in sys.path:
        sys.path.append(_p)

import ml_dtypes

B, S, D = 4, 2048, 1024
F = 4 * D
E = 8
TOP_K = 2
P = 128
CC = 512           # token chunk (free dim of matmuls; PSUM bank = 512 fp32)
KO = D // P        # 8  k-subtiles for the first matmul
FT = F // P        # 32 f-tiles (partition tiles of h)
DT = D // P        # 8  d-tiles (partition tiles of y)
FBLK = 512         # W1 wave width (f-columns per wave)
FB = F // FBLK     # 8 waves
G = 4              # W2 batches
GO = FT // G       # 8 fo-tiles per batch
WARM_N = 30        # PE warmup matmuls (HAM un-throttle) during startup DMA

# test-harness hooks (left off for grading)
TRACE = False
LAST_RESULTS = None

_compiled = {}


def _split_drain_waits(nc, max_waits=1):
    """This walrus build rejects instructions carrying more than one sync
    wait ("Too many sync wait commands"). Keep one wait on the instruction and
    move the excess onto NoOps inserted right before it on the same engine
    (engines are in-order, so blocking semantics are identical). Updates stay
    on the original instruction — moving them to a trailing NoOp could signal
    before the op's writes land."""
    import concourse.mybir as mybir

    m = nc.m
    new_module = copy.replace(m, functions=[])
    for function in m.functions:
        new_function = copy.replace(function, blocks=[])
        new_function.set_allocations_from_list(function.allocations)
        for block in function.blocks:
            out = []
            for inst in block.instructions:
                si = getattr(inst, "sync_info", None)
                on_wait = list(si.on_wait) if si is not None and si.on_wait else []
                if len(on_wait) > max_waits:
                    engine = getattr(inst, "engine", None)
                    extra, keep = on_wait[max_waits:], on_wait[:max_waits]
                    for j, w in enumerate(extra):
                        out.append(
                            mybir.InstNoOp(
                                name=f"{inst.name}-w{j}",
                                engine=engine,
                                sync_info=mybir.SyncInfo(on_wait=[w], on_update=[]),
                                bass_nofuse=True,
                            )
                        )
                    inst.sync_info = mybir.SyncInfo(
                        on_wait=keep,
                        on_update=list(si.on_update) if si.on_update else [],
                    )
                out.append(inst)
            new_function.blocks.append(copy.replace(block, instructions=out))
        new_module.functions.append(new_function)
    nc.m = new_module
    return nc


def _build_nc(C):
    import concourse.bass as bass
    import concourse.mybir as mybir
    from concourse.tile import TileContext

    fp32 = mybir.dt.float32
    bf16 = mybir.dt.bfloat16
    AF = mybir.ActivationFunctionType

    nch = -(-C // CC)
    chunks = [(i * CC, min(CC, C - i * CC)) for i in range(nch)]

    nc = bass.Bass()
    # All DRAM tensors are packed host-side so that one SBUF partition's data
    # is contiguous in DRAM (8-16KB per partition per transfer).
    xp = nc.declare_dram_parameter("xp", [nch, P, KO * CC], bf16, isOutput=False)
    w1p = nc.declare_dram_parameter("w1p", [FB, P, KO * FBLK], bf16, isOutput=False)
    w2p = nc.declare_dram_parameter("w2p", [G, P, GO * D], bf16, isOutput=False)
    b1 = nc.declare_dram_parameter("b1", [P, FT], fp32, isOutput=False)
    b2 = nc.declare_dram_parameter("b2", [P, DT], fp32, isOutput=False)
    yp = nc.declare_dram_parameter("yp", [nch, P, DT * CC], bf16, isOutput=True)

    with TileContext(nc) as tc:
        with (
            tc.tile_pool(name="wpool", bufs=1) as wpool,
            tc.tile_pool(name="xpool", bufs=2) as xpool,
            tc.tile_pool(name="hpool", bufs=1) as hpool,
            tc.tile_pool(name="ypool", bufs=2) as ypool,
            tc.tile_pool(name="hpsum", bufs=4, space="PSUM") as hpsum,
            tc.tile_pool(name="ypsum", bufs=4, space="PSUM") as ypsum,
        ):
            # DMA queue order == program order: chunk-0 activations + the
            # first W1 wave in half-transfers (the blockers for the first
            # real matmul), then the rest of the weights in deadline order.
            # The warm tile is memset on-device so the PE warmup loop can
            # start during the DMA ramp without waiting on any transfer.
            warm_sb = wpool.tile([P, CC], bf16, tag="warm")
            nc.vector.memset(warm_sb[:], 0.0)

            HK = KO // 2 * CC
            x_sb = [None] * nch
            x_sb[0] = xpool.tile([P, KO * CC], bf16, tag="x", name="x0")
            nc.sync.dma_start(x_sb[0][:, :HK], xp[0, :, :HK])

            HW = KO // 2 * FBLK
            w1_sb = [None] * FB
            w1_sb[0] = wpool.tile([P, KO * FBLK], bf16, tag="w1w0", name="w1s0")
            nc.sync.dma_start(w1_sb[0][:, :HW], w1p[0, :, :HW])

            b1_sb = wpool.tile([P, FT], fp32, tag="b1")
            nc.sync.dma_start(b1_sb[:], b1[:])

            nc.sync.dma_start(x_sb[0][:, HK:], xp[0, :, HK:])
            nc.sync.dma_start(w1_sb[0][:, HW:], w1p[0, :, HW:])

            b2_sb = wpool.tile([P, DT], fp32, tag="b2")
            nc.sync.dma_start(b2_sb[:], b2[:])

            for fb in range(1, FB):
                w1_sb[fb] = wpool.tile([P, KO * FBLK], bf16, tag=f"w1w{fb}",
                                       name=f"w1s{fb}")
                nc.sync.dma_start(w1_sb[fb][:], w1p[fb])
            w2_sb = [None] * G
            for g in range(G):
                w2_sb[g] = wpool.tile([P, GO * D], bf16, tag=f"w2g{g}",
                                      name=f"w2s{g}")
                nc.sync.dma_start(w2_sb[g][:], w2p[g])

            # PE warmup: full-size matmuls on the memset tile (no DMA deps)
            # keep the PE busy through the HAM activity window so the real
            # matmuls start at the full 2.4 GHz clock. Count is paced to end
            # roughly when the chunk-0 activations/weights land.
            warm_ps = hpsum.tile([P, CC], fp32, tag="hps")
            for _ in range(WARM_N):
                nc.tensor.matmul(
                    warm_ps[:, :], warm_sb[:, 0:P], warm_sb[:, :],
                    start=True, stop=True,
                )

            for ci, (c0, cn) in enumerate(chunks):
                if ci + 1 < nch:
                    x_sb[ci + 1] = xpool.tile([P, KO * CC], bf16, tag="x",
                                              name=f"x{ci + 1}")
                    nc.sync.dma_start(x_sb[ci + 1][:], xp[ci + 1])

                h_sb = hpool.tile([P, FT * CC], bf16, tag="h")
                for ft in range(FT):
                    fb, fc = divmod(ft * P, FBLK)
                    h_ps = hpsum.tile([P, CC], fp32, tag="hps")
                    for ko in range(KO):
                        nc.tensor.matmul(
                            h_ps[:, :cn],
                            w1_sb[fb][:, ko * FBLK + fc: ko * FBLK + fc + P],
                            x_sb[ci][:, ko * CC: ko * CC + cn],
                            start=(ko == 0),
                            stop=(ko == KO - 1),
                        )
                    # gelu(mm + b1) fused on ScalarE, cast to bf16 on write
                    nc.scalar.activation(
                        h_sb[:, ft * CC: ft * CC + cn], h_ps[:, :cn], AF.Gelu,
                        bias=b1_sb[:, ft:ft + 1],
                    )

                y_sb = ypool.tile([P, DT * CC], bf16, tag="y")
                for dt_ in range(DT):
                    y_ps = ypsum.tile([P, CC], fp32, tag="yps")
                    for fo in range(FT):
                        g, gl = divmod(fo, GO)
                        nc.tensor.matmul(
                            y_ps[:, :cn],
                            w2_sb[g][:, gl * D + dt_ * P: gl * D + dt_ * P + P],
                            h_sb[:, fo * CC: fo * CC + cn],
                            start=(fo == 0),
                            stop=(fo == FT - 1),
                        )
                    nc.vector.tensor_scalar_add(
                        y_sb[:, dt_ * CC: dt_ * CC + cn], y_ps[:, :cn],
                        b2_sb[:, dt_:dt_ + 1],
                    )
                if cn == CC:
                    nc.sync.dma_start(yp[ci], y_sb[:])
                else:
                    for dt_ in range(DT):
                        nc.sync.dma_start(
                            yp[ci, :, dt_ * CC: dt_ * CC + cn],
                            y_sb[:, dt_ * CC: dt_ * CC + cn],
                        )

    return _split_drain_waits(nc)


def _to_bf16(a):
    """Fast float32 -> bfloat16 with round-to-nearest-even via bit ops."""
    a = np.ascontiguousarray(a, dtype=np.float32)
    u = a.view(np.uint32)
    r = ((u + 0x7FFF + ((u >> 16) & 1)) >> 16).astype(np.uint16)
    return r.view(ml_dtypes.bfloat16)


def kernel(hidden_states, Wg, bg, W1, b1, W2, b2):
    from concourse import bass_utils

    hs = np.ascontiguousarray(hidden_states, dtype=np.float32).reshape(B * S, D)

    # ---- Gate on host (float64): softmax over experts, top-2, renormalize
    logits = hs.astype(np.float64) @ np.asarray(Wg, np.float64).T
    logits += np.asarray(bg, np.float64)
    logits -= logits.max(axis=-1, keepdims=True)
    p = np.exp(logits)
    p /= p.sum(axis=-1, keepdims=True)

    i1 = p.argmax(axis=-1)
    rows = np.arange(B * S)
    p1 = p[rows, i1]
    pm = p.copy()
    pm[rows, i1] = -1.0
    i2 = pm.argmax(axis=-1)
    p2 = p[rows, i2]
    denom = p1 + p2
    g1 = (p1 / denom).astype(np.float32)
    g2 = (p2 / denom).astype(np.float32)

    # ---- Dispatch: token ids + combine weights per expert
    ids, cws = [], []
    for e in range(E):
        m1 = np.nonzero(i1 == e)[0]
        m2 = np.nonzero(i2 == e)[0]
        ids.append(np.concatenate([m1, m2]))
        cws.append(np.concatenate([g1[m1], g2[m2]]))
    max_cnt = max(len(x) for x in ids)
    C = max(P, -(-max_cnt // P) * P)
    nch = -(-C // CC)

    if C not in _compiled:
        _compiled[C] = _build_nc(C)
    nc = _compiled[C]

    in_maps = []
    for e in range(E):
        xT = np.zeros((D, nch * CC), dtype=ml_dtypes.bfloat16)
        cnt = len(ids[e])
        xT[:, :cnt] = _to_bf16(hs[ids[e]]).T
        # pack: xp[ch, ki, ko*CC + c'] = xT[ko*P + ki, ch*CC + c']
        xpk = np.ascontiguousarray(
            xT.reshape(KO, P, nch, CC).transpose(2, 1, 0, 3).reshape(nch, P, KO * CC))
        # pack: w1p[fb, ki, ko*FBLK + f'] = W1[ko*P + ki, fb*FBLK + f']
        w1pk = np.ascontiguousarray(
            _to_bf16(W1[e]).reshape(KO, P, FB, FBLK)
            .transpose(2, 1, 0, 3).reshape(FB, P, KO * FBLK))
        # pack: w2p[g, fi, gl*D + d] = W2[(g*GO + gl)*P + fi, d]
        w2pk = np.ascontiguousarray(
            _to_bf16(W2[e]).reshape(G, GO, P, D)
            .transpose(0, 2, 1, 3).reshape(G, P, GO * D))
        in_maps.append({
            "xp": xpk,
            "w1p": w1pk,
            "w2p": w2pk,
            "b1": np.ascontiguousarray(
                np.asarray(b1[e], np.float32).reshape(FT, P).T),
            "b2": np.ascontiguousarray(
                np.asarray(b2[e], np.float32).reshape(DT, P).T),
        })

    kwargs = {}
    if TRACE:
        import os as _os
        kwargs = dict(trace=True, trace_cores=list(range(E)))
        if _os.environ.get("MOE_TRACE_DIR"):
            _os.makedirs(_os.environ["MOE_TRACE_DIR"], exist_ok=True)
            kwargs["tmpdir"] = _os.environ["MOE_TRACE_DIR"]
    res = bass_utils.run_bass_kernel_spmd(nc, in_maps, list(range(E)), **kwargs)
    global LAST_RESULTS
    LAST_RESULTS = res

    out = np.zeros((B * S, D), dtype=np.float32)
    for e in range(E):
        cnt = len(ids[e])
        # unpack: yp[ch, p, dt*CC + c'] = y[dt*P + p, ch*CC + c']
        ypk = np.asarray(res.results[e]["yp"], dtype=np.float32)
        yT = ypk.reshape(nch, P, DT, CC).transpose(2, 1, 0, 3).reshape(D, nch * CC)
        out[ids[e]] += cws[e][:, None] * yT[:, :cnt].T
    return out.reshape(B, S, D)


# revision 11
# speedup vs baseline: 1.0119x; 1.0119x over previous
"""Mixture-of-Experts (B=4, S=2048, D=1024, F=4096, E=8, top-2) on 8 trn2 NeuronCores.

Strategy: expert parallelism, one expert per core.
  - Host: gate (softmax + top-2 + renorm) in float64, dispatch (gather) tokens
    per expert, pad to a common capacity C, pack all device tensors so every
    DMA moves 8-16KB contiguous per SBUF partition (big-packet DMA).
  - Device (SPMD, identical program, per-core data): y^T = W2^T @ gelu(W1^T @ x^T + b1) + b2
    with both weights resident in SBUF as bf16 and tokens streamed in chunks
    of 512. PSUM accumulates over the contraction (D resp. F) in fp32.
    ~100 tiny warmup matmuls run during the initial weight DMA so the PE HAM
    clock-gate is already at 8/8 when the real matmuls start.
  - Host: combine with the gate weights (y *= cw) and scatter-add back into
    the [B*S, D] output. Token index sets are unique per expert, so fancy-index
    add per expert is race-free.
"""

import copy
import sys

import numpy as np

for _p in ("/opt/trn_rl_repo", "/opt/pypackages"):
    if _p not in sys.path:
        sys.path.append(_p)

import ml_dtypes

B, S, D = 4, 2048, 1024
F = 4 * D
E = 8
TOP_K = 2
P = 128
CC = 512           # token chunk (free dim of matmuls; PSUM bank = 512 fp32)
KO = D // P        # 8  k-subtiles for the first matmul
FT = F // P        # 32 f-tiles (partition tiles of h)
DT = D // P        # 8  d-tiles (partition tiles of y)
FBLK = 512         # W1 wave width (f-columns per wave)
FB = F // FBLK     # 8 waves
G = 4              # W2 batches
GO = FT // G       # 8 fo-tiles per batch
WARM_N = 80        # PE warmup matmuls (HAM un-throttle) during startup DMA

# test-harness hooks (left off for grading)
TRACE = False
LAST_RESULTS = None

_compiled = {}


def _split_drain_waits(nc, max_waits=1):
    """This walrus build rejects instructions carrying more than one sync
    wait ("Too many sync wait commands"). Keep one wait on the instruction and
    move the excess onto NoOps inserted right before it on the same engine
    (engines are in-order, so blocking semantics are identical). Updates stay
    on the original instruction — moving them to a trailing NoOp could signal
    before the op's writes land."""
    import concourse.mybir as mybir

    m = nc.m
    new_module = copy.replace(m, functions=[])
    for function in m.functions:
        new_function = copy.replace(function, blocks=[])
        new_function.set_allocations_from_list(function.allocations)
        for block in function.blocks:
            out = []
            for inst in block.instructions:
                si = getattr(inst, "sync_info", None)
                on_wait = list(si.on_wait) if si is not None and si.on_wait else []
                if len(on_wait) > max_waits:
                    engine = getattr(inst, "engine", None)
                    extra, keep = on_wait[max_waits:], on_wait[:max_waits]
                    for j, w in enumerate(extra):
                        out.append(
                            mybir.InstNoOp(
                                name=f"{inst.name}-w{j}",
                                engine=engine,
                                sync_info=mybir.SyncInfo(on_wait=[w], on_update=[]),
                                bass_nofuse=True,
                            )
                        )
                    inst.sync_info = mybir.SyncInfo(
                        on_wait=keep,
                        on_update=list(si.on_update) if si.on_update else [],
                    )
                out.append(inst)
            new_function.blocks.append(copy.replace(block, instructions=out))
        new_module.functions.append(new_function)
    nc.m = new_module
    return nc


def _build_nc(C):
    import concourse.bass as bass
    import concourse.mybir as mybir
    from concourse.tile import TileContext

    fp32 = mybir.dt.float32
    bf16 = mybir.dt.bfloat16
    AF = mybir.ActivationFunctionType

    nch = -(-C // CC)
    chunks = [(i * CC, min(CC, C - i * CC)) for i in range(nch)]

    nc = bass.Bass()
    # All DRAM tensors are packed host-side so that one SBUF partition's data
    # is contiguous in DRAM (8-16KB per partition per transfer).
    xp = nc.declare_dram_parameter("xp", [nch, P, KO * CC], bf16, isOutput=False)
    w1p = nc.declare_dram_parameter("w1p", [FB, P, KO * FBLK], bf16, isOutput=False)
    w2p = nc.declare_dram_parameter("w2p", [G, P, GO * D], bf16, isOutput=False)
    b1 = nc.declare_dram_parameter("b1", [P, FT], fp32, isOutput=False)
    b2 = nc.declare_dram_parameter("b2", [P, DT], fp32, isOutput=False)
    yp = nc.declare_dram_parameter("yp", [nch, P, DT * CC], bf16, isOutput=True)

    with TileContext(nc) as tc:
        with (
            tc.tile_pool(name="wpool", bufs=1) as wpool,
            tc.tile_pool(name="xpool", bufs=2) as xpool,
            tc.tile_pool(name="hpool", bufs=1) as hpool,
            tc.tile_pool(name="ypool", bufs=2) as ypool,
            tc.tile_pool(name="hpsum", bufs=4, space="PSUM") as hpsum,
            tc.tile_pool(name="ypsum", bufs=4, space="PSUM") as ypsum,
        ):
            # DMA queue order == program order: chunk-0 activations + the
            # first W1 wave in half-transfers (the blockers for the first
            # real matmul), then the rest of the weights in deadline order.
            # The warm tile is memset on-device so the PE warmup loop can
            # start during the DMA ramp without waiting on any transfer.
            warm_sb = wpool.tile([P, CC], bf16, tag="warm")
            nc.vector.memset(warm_sb[:], 0.0)

            # chunk-0 x and W1-wave-0 arrive in ko-pair quarters so the first
            # mm1 chain can start after only ~0.5MB has landed.
            x_sb = [None] * nch
            x_sb[0] = xpool.tile([P, KO * CC], bf16, tag="x", name="x0")
            w1_sb = [None] * FB
            w1_sb[0] = wpool.tile([P, KO * FBLK], bf16, tag="w1w0", name="w1s0")
            for j in range(4):
                nc.sync.dma_start(
                    x_sb[0][:, 2 * j * CC: 2 * (j + 1) * CC],
                    xp[0, :, 2 * j * CC: 2 * (j + 1) * CC])
                nc.sync.dma_start(
                    w1_sb[0][:, 2 * j * FBLK: 2 * (j + 1) * FBLK],
                    w1p[0, :, 2 * j * FBLK: 2 * (j + 1) * FBLK])
                if j == 0:
                    b1_sb = wpool.tile([P, FT], fp32, tag="b1")
                    nc.sync.dma_start(b1_sb[:], b1[:])

            b2_sb = wpool.tile([P, DT], fp32, tag="b2")
            nc.sync.dma_start(b2_sb[:], b2[:])

            for fb in range(1, FB):
                w1_sb[fb] = wpool.tile([P, KO * FBLK], bf16, tag=f"w1w{fb}",
                                       name=f"w1s{fb}")
                nc.sync.dma_start(w1_sb[fb][:], w1p[fb])
            w2_sb = [None] * G
            for g in range(G):
                w2_sb[g] = wpool.tile([P, GO * D], bf16, tag=f"w2g{g}",
                                      name=f"w2s{g}")
                nc.sync.dma_start(w2_sb[g][:], w2p[g])

            # PE warmup: full-size matmuls on the memset tile (no DMA deps)
            # keep the PE busy through the HAM activity window so the real
            # matmuls start at the full 2.4 GHz clock. Count is paced to end
            # roughly when the chunk-0 activations/weights land.
            warm_ps = hpsum.tile([P, CC], fp32, tag="hps")
            for _ in range(WARM_N):
                nc.tensor.matmul(
                    warm_ps[:, 0:64], warm_sb[:, 0:P], warm_sb[:, 0:64],
                    start=True, stop=True,
                )

            for ci, (c0, cn) in enumerate(chunks):
                if ci + 1 < nch:
                    x_sb[ci + 1] = xpool.tile([P, KO * CC], bf16, tag="x",
                                              name=f"x{ci + 1}")
                    nc.sync.dma_start(x_sb[ci + 1][:], xp[ci + 1])

                h_sb = hpool.tile([P, FT * CC], bf16, tag="h")
                for ft in range(FT):
                    fb, fc = divmod(ft * P, FBLK)
                    h_ps = hpsum.tile([P, CC], fp32, tag="hps")
                    for ko in range(KO):
                        nc.tensor.matmul(
                            h_ps[:, :cn],
                            w1_sb[fb][:, ko * FBLK + fc: ko * FBLK + fc + P],
                            x_sb[ci][:, ko * CC: ko * CC + cn],
                            start=(ko == 0),
                            stop=(ko == KO - 1),
                        )
                    # gelu(mm + b1) fused on ScalarE, cast to bf16 on write
                    nc.scalar.activation(
                        h_sb[:, ft * CC: ft * CC + cn], h_ps[:, :cn], AF.Gelu,
                        bias=b1_sb[:, ft:ft + 1],
                    )

                y_sb = ypool.tile([P, DT * CC], bf16, tag="y")
                for dt_ in range(DT):
                    y_ps = ypsum.tile([P, CC], fp32, tag="yps")
                    for fo in range(FT):
                        g, gl = divmod(fo, GO)
                        nc.tensor.matmul(
                            y_ps[:, :cn],
                            w2_sb[g][:, gl * D + dt_ * P: gl * D + dt_ * P + P],
                            h_sb[:, fo * CC: fo * CC + cn],
                            start=(fo == 0),
                            stop=(fo == FT - 1),
                        )
                    nc.vector.tensor_scalar_add(
                        y_sb[:, dt_ * CC: dt_ * CC + cn], y_ps[:, :cn],
                        b2_sb[:, dt_:dt_ + 1],
                    )
                if cn == CC:
                    nc.sync.dma_start(yp[ci], y_sb[:])
                else:
                    for dt_ in range(DT):
                        nc.sync.dma_start(
                            yp[ci, :, dt_ * CC: dt_ * CC + cn],
                            y_sb[:, dt_ * CC: dt_ * CC + cn],
                        )

    return _split_drain_waits(nc)


def _to_bf16(a):
    """Fast float32 -> bfloat16 with round-to-nearest-even via bit ops."""
    a = np.ascontiguousarray(a, dtype=np.float32)
    u = a.view(np.uint32)
    r = ((u + 0x7FFF + ((u >> 16) & 1)) >> 16).astype(np.uint16)
    return r.view(ml_dtypes.bfloat16)


def kernel(hidden_states, Wg, bg, W1, b1, W2, b2):
    from concourse import bass_utils

    hs = np.ascontiguousarray(hidden_states, dtype=np.float32).reshape(B * S, D)

    # ---- Gate on host (float64): softmax over experts, top-2, renormalize
    logits = hs.astype(np.float64) @ np.asarray(Wg, np.float64).T
    logits += np.asarray(bg, np.float64)
    logits -= logits.max(axis=-1, keepdims=True)
    p = np.exp(logits)
    p /= p.sum(axis=-1, keepdims=True)

    i1 = p.argmax(axis=-1)
    rows = np.arange(B * S)
    p1 = p[rows, i1]
    pm = p.copy()
    pm[rows, i1] = -1.0
    i2 = pm.argmax(axis=-1)
    p2 = p[rows, i2]
    denom = p1 + p2
    g1 = (p1 / denom).astype(np.float32)
    g2 = (p2 / denom).astype(np.float32)

    # ---- Dispatch: token ids + combine weights per expert
    ids, cws = [], []
    for e in range(E):
        m1 = np.nonzero(i1 == e)[0]
        m2 = np.nonzero(i2 == e)[0]
        ids.append(np.concatenate([m1, m2]))
        cws.append(np.concatenate([g1[m1], g2[m2]]))
    max_cnt = max(len(x) for x in ids)
    C = max(P, -(-max_cnt // P) * P)
    nch = -(-C // CC)

    if C not in _compiled:
        _compiled[C] = _build_nc(C)
    nc = _compiled[C]

    in_maps = []
    for e in range(E):
        xT = np.zeros((D, nch * CC), dtype=ml_dtypes.bfloat16)
        cnt = len(ids[e])
        xT[:, :cnt] = _to_bf16(hs[ids[e]]).T
        # pack: xp[ch, ki, ko*CC + c'] = xT[ko*P + ki, ch*CC + c']
        xpk = np.ascontiguousarray(
            xT.reshape(KO, P, nch, CC).transpose(2, 1, 0, 3).reshape(nch, P, KO * CC))
        # pack: w1p[fb, ki, ko*FBLK + f'] = W1[ko*P + ki, fb*FBLK + f']
        w1pk = np.ascontiguousarray(
            _to_bf16(W1[e]).reshape(KO, P, FB, FBLK)
            .transpose(2, 1, 0, 3).reshape(FB, P, KO * FBLK))
        # pack: w2p[g, fi, gl*D + d] = W2[(g*GO + gl)*P + fi, d]
        w2pk = np.ascontiguousarray(
            _to_bf16(W2[e]).reshape(G, GO, P, D)
            .transpose(0, 2, 1, 3).reshape(G, P, GO * D))
        in_maps.append({
            "xp": xpk,
            "w1p": w1pk,
            "w2p": w2pk,
            "b1": np.ascontiguousarray(
                np.asarray(b1[e], np.float32).reshape(FT, P).T),
            "b2": np.ascontiguousarray(
                np.asarray(b2[e], np.float32).reshape(DT, P).T),
        })

    kwargs = {}
    if TRACE:
        import os as _os
        kwargs = dict(trace=True, trace_cores=list(range(E)))
        if _os.environ.get("MOE_TRACE_DIR"):
            _os.makedirs(_os.environ["MOE_TRACE_DIR"], exist_ok=True)
            kwargs["tmpdir"] = _os.environ["MOE_TRACE_DIR"]
    res = bass_utils.run_bass_kernel_spmd(nc, in_maps, list(range(E)), **kwargs)
    global LAST_RESULTS
    LAST_RESULTS = res

    out = np.zeros((B * S, D), dtype=np.float32)
    for e in range(E):
        cnt = len(ids[e])
        # unpack: yp[ch, p, dt*CC + c'] = y[dt*P + p, ch*CC + c']
        ypk = np.asarray(res.results[e]["yp"], dtype=np.float32)
        yT = ypk.reshape(nch, P, DT, CC).transpose(2, 1, 0, 3).reshape(D, nch * CC)
        out[ids[e]] += cws[e][:, None] * yT[:, :cnt].T
    return out.reshape(B, S, D)


# revision 13
# speedup vs baseline: 1.0120x; 1.0002x over previous
"""Mixture-of-Experts (B=4, S=2048, D=1024, F=4096, E=8, top-2) on 8 trn2 NeuronCores.

Strategy: expert parallelism, one expert per core.
  - Host: gate (softmax + top-2 + renorm) in float64, dispatch (gather) tokens
    per expert, pad to a common capacity C, pack all device tensors so every
    DMA moves 8-16KB contiguous per SBUF partition (big-packet DMA).
  - Device (SPMD, identical program, per-core data): y^T = W2^T @ gelu(W1^T @ x^T + b1) + b2
    with both weights resident in SBUF as bf16 and tokens streamed in chunks
    of 512. PSUM accumulates over the contraction (D resp. F) in fp32.
    ~100 tiny warmup matmuls run during the initial weight DMA so the PE HAM
    clock-gate is already at 8/8 when the real matmuls start.
  - Host: combine with the gate weights (y *= cw) and scatter-add back into
    the [B*S, D] output. Token index sets are unique per expert, so fancy-index
    add per expert is race-free.
"""

import copy
import sys

import numpy as np

for _p in ("/opt/trn_rl_repo", "/opt/pypackages"):
    if _p not in sys.path:
        sys.path.append(_p)

import ml_dtypes

B, S, D = 4, 2048, 1024
F = 4 * D
E = 8
TOP_K = 2
P = 128
CC = 512           # token chunk (free dim of matmuls; PSUM bank = 512 fp32)
KO = D // P        # 8  k-subtiles for the first matmul
FT = F // P        # 32 f-tiles (partition tiles of h)
DT = D // P        # 8  d-tiles (partition tiles of y)
FBLK = 512         # W1 wave width (f-columns per wave)
FB = F // FBLK     # 8 waves
G = 4              # W2 batches
GO = FT // G       # 8 fo-tiles per batch
WARM_N = 68        # PE warmup matmuls (HAM un-throttle) during startup DMA

# test-harness hooks (left off for grading)
TRACE = False
LAST_RESULTS = None

_compiled = {}


def _split_drain_waits(nc, max_waits=1):
    """This walrus build rejects instructions carrying more than one sync
    wait ("Too many sync wait commands"). Keep one wait on the instruction and
    move the excess onto NoOps inserted right before it on the same engine
    (engines are in-order, so blocking semantics are identical). Updates stay
    on the original instruction — moving them to a trailing NoOp could signal
    before the op's writes land."""
    import concourse.mybir as mybir

    m = nc.m
    new_module = copy.replace(m, functions=[])
    for function in m.functions:
        new_function = copy.replace(function, blocks=[])
        new_function.set_allocations_from_list(function.allocations)
        for block in function.blocks:
            out = []
            for inst in block.instructions:
                si = getattr(inst, "sync_info", None)
                on_wait = list(si.on_wait) if si is not None and si.on_wait else []
                if len(on_wait) > max_waits:
                    engine = getattr(inst, "engine", None)
                    extra, keep = on_wait[max_waits:], on_wait[:max_waits]
                    for j, w in enumerate(extra):
                        out.append(
                            mybir.InstNoOp(
                                name=f"{inst.name}-w{j}",
                                engine=engine,
                                sync_info=mybir.SyncInfo(on_wait=[w], on_update=[]),
                                bass_nofuse=True,
                            )
                        )
                    inst.sync_info = mybir.SyncInfo(
                        on_wait=keep,
                        on_update=list(si.on_update) if si.on_update else [],
                    )
                out.append(inst)
            new_function.blocks.append(copy.replace(block, instructions=out))
        new_module.functions.append(new_function)
    nc.m = new_module
    return nc


def _build_nc(C):
    import concourse.bass as bass
    import concourse.mybir as mybir
    from concourse.tile import TileContext

    fp32 = mybir.dt.float32
    bf16 = mybir.dt.bfloat16
    AF = mybir.ActivationFunctionType

    nch = -(-C // CC)
    chunks = [(i * CC, min(CC, C - i * CC)) for i in range(nch)]

    nc = bass.Bass()
    # All DRAM tensors are packed host-side so that one SBUF partition's data
    # is contiguous in DRAM (8-16KB per partition per transfer).
    xp = nc.declare_dram_parameter("xp", [nch, P, KO * CC], bf16, isOutput=False)
    w1p = nc.declare_dram_parameter("w1p", [FB, P, KO * FBLK], bf16, isOutput=False)
    w2p = nc.declare_dram_parameter("w2p", [G, P, GO * D], bf16, isOutput=False)
    b1 = nc.declare_dram_parameter("b1", [P, FT], fp32, isOutput=False)
    b2 = nc.declare_dram_parameter("b2", [P, DT], fp32, isOutput=False)
    yp = nc.declare_dram_parameter("yp", [nch, P, DT * CC], bf16, isOutput=True)

    with TileContext(nc) as tc:
        with (
            tc.tile_pool(name="wpool", bufs=1) as wpool,
            tc.tile_pool(name="xpool", bufs=2) as xpool,
            tc.tile_pool(name="hpool", bufs=1) as hpool,
            tc.tile_pool(name="ypool", bufs=2) as ypool,
            tc.tile_pool(name="hpsum", bufs=4, space="PSUM") as hpsum,
            tc.tile_pool(name="ypsum", bufs=4, space="PSUM") as ypsum,
        ):
            # DMA queue order == program order: chunk-0 activations + the
            # first W1 wave in half-transfers (the blockers for the first
            # real matmul), then the rest of the weights in deadline order.
            # The warm tile is memset on-device so the PE warmup loop can
            # start during the DMA ramp without waiting on any transfer.
            warm_sb = wpool.tile([P, P], bf16, tag="warm")
            nc.vector.memset(warm_sb[:], 0.0)

            # chunk-0 x and W1-wave-0 arrive in ko-pair quarters so the first
            # mm1 chain can start after only ~0.5MB has landed.
            x_sb = [None] * nch
            x_sb[0] = xpool.tile([P, KO * CC], bf16, tag="x", name="x0")
            w1_sb = [None] * FB
            w1_sb[0] = wpool.tile([P, KO * FBLK], bf16, tag="w1w0", name="w1s0")
            for j in range(4):
                nc.sync.dma_start(
                    x_sb[0][:, 2 * j * CC: 2 * (j + 1) * CC],
                    xp[0, :, 2 * j * CC: 2 * (j + 1) * CC])
                nc.sync.dma_start(
                    w1_sb[0][:, 2 * j * FBLK: 2 * (j + 1) * FBLK],
                    w1p[0, :, 2 * j * FBLK: 2 * (j + 1) * FBLK])
                if j == 0:
                    b1_sb = wpool.tile([P, FT], fp32, tag="b1")
                    nc.sync.dma_start(b1_sb[:], b1[:])

            b2_sb = wpool.tile([P, DT], fp32, tag="b2")
            nc.sync.dma_start(b2_sb[:], b2[:])

            for fb in range(1, FB):
                w1_sb[fb] = wpool.tile([P, KO * FBLK], bf16, tag=f"w1w{fb}",
                                       name=f"w1s{fb}")
                nc.sync.dma_start(w1_sb[fb][:], w1p[fb])
            w2_sb = [None] * G
            for g in range(G):
                w2_sb[g] = wpool.tile([P, GO * D], bf16, tag=f"w2g{g}",
                                      name=f"w2s{g}")
                nc.sync.dma_start(w2_sb[g][:], w2p[g])

            # PE warmup: full-size matmuls on the memset tile (no DMA deps)
            # keep the PE busy through the HAM activity window so the real
            # matmuls start at the full 2.4 GHz clock. Count is paced to end
            # roughly when the chunk-0 activations/weights land.
            warm_ps = hpsum.tile([P, CC], fp32, tag="hps")
            for _ in range(WARM_N):
                nc.tensor.matmul(
                    warm_ps[:, 0:64], warm_sb[:, 0:P], warm_sb[:, 0:64],
                    start=True, stop=True,
                )

            for ci, (c0, cn) in enumerate(chunks):
                if ci + 1 < nch:
                    x_sb[ci + 1] = xpool.tile([P, KO * CC], bf16, tag="x",
                                              name=f"x{ci + 1}")
                    nc.sync.dma_start(x_sb[ci + 1][:], xp[ci + 1])

                h_sb = hpool.tile([P, FT * CC], bf16, tag="h")
                for ft in range(FT):
                    fb, fc = divmod(ft * P, FBLK)
                    h_ps = hpsum.tile([P, CC], fp32, tag="hps")
                    for ko in range(KO):
                        nc.tensor.matmul(
                            h_ps[:, :cn],
                            w1_sb[fb][:, ko * FBLK + fc: ko * FBLK + fc + P],
                            x_sb[ci][:, ko * CC: ko * CC + cn],
                            start=(ko == 0),
                            stop=(ko == KO - 1),
                        )
                    # gelu(mm + b1) fused on ScalarE, cast to bf16 on write
                    nc.scalar.activation(
                        h_sb[:, ft * CC: ft * CC + cn], h_ps[:, :cn], AF.Gelu,
                        bias=b1_sb[:, ft:ft + 1],
                    )

                y_sb = ypool.tile([P, DT * CC], bf16, tag="y")
                for dt_ in range(DT):
                    y_ps = ypsum.tile([P, CC], fp32, tag="yps")
                    for fo in range(FT):
                        g, gl = divmod(fo, GO)
                        nc.tensor.matmul(
                            y_ps[:, :cn],
                            w2_sb[g][:, gl * D + dt_ * P: gl * D + dt_ * P + P],
                            h_sb[:, fo * CC: fo * CC + cn],
                            start=(fo == 0),
                            stop=(fo == FT - 1),
                        )
                    nc.vector.tensor_scalar_add(
                        y_sb[:, dt_ * CC: dt_ * CC + cn], y_ps[:, :cn],
                        b2_sb[:, dt_:dt_ + 1],
                    )
                if cn == CC:
                    nc.sync.dma_start(yp[ci], y_sb[:])
                else:
                    for dt_ in range(DT):
                        nc.sync.dma_start(
                            yp[ci, :, dt_ * CC: dt_ * CC + cn],
                            y_sb[:, dt_ * CC: dt_ * CC + cn],
                        )

    return _split_drain_waits(nc)


def _to_bf16(a):
    """Fast float32 -> bfloat16 with round-to-nearest-even via bit ops."""
    a = np.ascontiguousarray(a, dtype=np.float32)
    u = a.view(np.uint32)
    r = ((u + 0x7FFF + ((u >> 16) & 1)) >> 16).astype(np.uint16)
    return r.view(ml_dtypes.bfloat16)


def kernel(hidden_states, Wg, bg, W1, b1, W2, b2):
    from concourse import bass_utils

    hs = np.ascontiguousarray(hidden_states, dtype=np.float32).reshape(B * S, D)

    # ---- Gate on host (float64): softmax over experts, top-2, renormalize
    logits = hs.astype(np.float64) @ np.asarray(Wg, np.float64).T
    logits += np.asarray(bg, np.float64)
    logits -= logits.max(axis=-1, keepdims=True)
    p = np.exp(logits)
    p /= p.sum(axis=-1, keepdims=True)

    i1 = p.argmax(axis=-1)
    rows = np.arange(B * S)
    p1 = p[rows, i1]
    pm = p.copy()
    pm[rows, i1] = -1.0
    i2 = pm.argmax(axis=-1)
    p2 = p[rows, i2]
    denom = p1 + p2
    g1 = (p1 / denom).astype(np.float32)
    g2 = (p2 / denom).astype(np.float32)

    # ---- Dispatch: token ids + combine weights per expert
    ids, cws = [], []
    for e in range(E):
        m1 = np.nonzero(i1 == e)[0]
        m2 = np.nonzero(i2 == e)[0]
        ids.append(np.concatenate([m1, m2]))
        cws.append(np.concatenate([g1[m1], g2[m2]]))
    max_cnt = max(len(x) for x in ids)
    C = max(P, -(-max_cnt // P) * P)
    nch = -(-C // CC)

    if C not in _compiled:
        _compiled[C] = _build_nc(C)
    nc = _compiled[C]

    in_maps = []
    for e in range(E):
        xT = np.zeros((D, nch * CC), dtype=ml_dtypes.bfloat16)
        cnt = len(ids[e])
        xT[:, :cnt] = _to_bf16(hs[ids[e]]).T
        # pack: xp[ch, ki, ko*CC + c'] = xT[ko*P + ki, ch*CC + c']
        xpk = np.ascontiguousarray(
            xT.reshape(KO, P, nch, CC).transpose(2, 1, 0, 3).reshape(nch, P, KO * CC))
        # pack: w1p[fb, ki, ko*FBLK + f'] = W1[ko*P + ki, fb*FBLK + f']
        w1pk = np.ascontiguousarray(
            _to_bf16(W1[e]).reshape(KO, P, FB, FBLK)
            .transpose(2, 1, 0, 3).reshape(FB, P, KO * FBLK))
        # pack: w2p[g, fi, gl*D + d] = W2[(g*GO + gl)*P + fi, d]
        w2pk = np.ascontiguousarray(
            _to_bf16(W2[e]).reshape(G, GO, P, D)
            .transpose(0, 2, 1, 3).reshape(G, P, GO * D))
        in_maps.append({
            "xp": xpk,
            "w1p": w1pk,
            "w2p": w2pk,
            "b1": np.ascontiguousarray(
                np.asarray(b1[e], np.float32).reshape(FT, P).T),
            "b2": np.ascontiguousarray(
                np.asarray(b2[e], np.float32).reshape(DT, P).T),
        })

    kwargs = {}
    if TRACE:
        import os as _os
        kwargs = dict(trace=True, trace_cores=list(range(E)))
        if _os.environ.get("MOE_TRACE_DIR"):
            _os.makedirs(_os.environ["MOE_TRACE_DIR"], exist_ok=True)
            kwargs["tmpdir"] = _os.environ["MOE_TRACE_DIR"]
    res = bass_utils.run_bass_kernel_spmd(nc, in_maps, list(range(E)), **kwargs)
    global LAST_RESULTS
    LAST_RESULTS = res

    out = np.zeros((B * S, D), dtype=np.float32)
    for e in range(E):
        cnt = len(ids[e])
        # unpack: yp[ch, p, dt*CC + c'] = y[dt*P + p, ch*CC + c']
        ypk = np.asarray(res.results[e]["yp"], dtype=np.float32)
        yT = ypk.reshape(nch, P, DT, CC).transpose(2, 1, 0, 3).reshape(D, nch * CC)
        out[ids[e]] += cws[e][:, None] * yT[:, :cnt].T
    return out.reshape(B, S, D)


# revision 14
# speedup vs baseline: 1.0136x; 1.0016x over previous
"""Mixture-of-Experts (B=4, S=2048, D=1024, F=4096, E=8, top-2) on 8 trn2 NeuronCores.

Strategy: expert parallelism, one expert per core.
  - Host: gate (softmax + top-2 + renorm) in float64, dispatch (gather) tokens
    per expert, pad to a common capacity C, pack all device tensors so every
    DMA moves 8-16KB contiguous per SBUF partition (big-packet DMA).
  - Device (SPMD, identical program, per-core data): y^T = W2^T @ gelu(W1^T @ x^T + b1) + b2
    with both weights resident in SBUF as bf16 and tokens streamed in chunks
    of 512. PSUM accumulates over the contraction (D resp. F) in fp32.
    A paced run of warmup matmuls on a memset tile covers the initial DMA
    window so the PE HAM clock-gate is already at 8/8 when real matmuls start.
  - Host: combine with the gate weights (y *= cw) and scatter-add back into
    the [B*S, D] output. Token index sets are unique per expert, so fancy-index
    add per expert is race-free.
"""

import copy
import sys

import numpy as np

for _p in ("/opt/trn_rl_repo", "/opt/pypackages"):
    if _p not in sys.path:
        sys.path.append(_p)

import ml_dtypes

B, S, D = 4, 2048, 1024
F = 4 * D
E = 8
TOP_K = 2
P = 128
CC = 512           # token chunk (free dim of matmuls; PSUM bank = 512 fp32)
KO = D // P        # 8  k-subtiles for the first matmul
FT = F // P        # 32 f-tiles (partition tiles of h)
DT = D // P        # 8  d-tiles (partition tiles of y)
FBLK = 512         # W1 wave width (f-columns per wave)
FB = F // FBLK     # 8 waves
G = 4              # W2 batches
GO = FT // G       # 8 fo-tiles per batch
WARM_N = 68        # PE warmup matmuls (HAM un-throttle) during startup DMA

# test-harness hooks (left off for grading)
TRACE = False
LAST_RESULTS = None

_compiled = {}


def _split_drain_waits(nc, max_waits=1):
    """This walrus build rejects instructions carrying more than one sync
    wait ("Too many sync wait commands"). Keep one wait on the instruction and
    move the excess onto NoOps inserted right before it on the same engine
    (engines are in-order, so blocking semantics are identical). Updates stay
    on the original instruction — moving them to a trailing NoOp could signal
    before the op's writes land."""
    import concourse.mybir as mybir

    m = nc.m
    new_module = copy.replace(m, functions=[])
    for function in m.functions:
        new_function = copy.replace(function, blocks=[])
        new_function.set_allocations_from_list(function.allocations)
        for block in function.blocks:
            out = []
            for inst in block.instructions:
                si = getattr(inst, "sync_info", None)
                on_wait = list(si.on_wait) if si is not None and si.on_wait else []
                if len(on_wait) > max_waits:
                    engine = getattr(inst, "engine", None)
                    extra, keep = on_wait[max_waits:], on_wait[:max_waits]
                    for j, w in enumerate(extra):
                        out.append(
                            mybir.InstNoOp(
                                name=f"{inst.name}-w{j}",
                                engine=engine,
                                sync_info=mybir.SyncInfo(on_wait=[w], on_update=[]),
                                bass_nofuse=True,
                            )
                        )
                    inst.sync_info = mybir.SyncInfo(
                        on_wait=keep,
                        on_update=list(si.on_update) if si.on_update else [],
                    )
                out.append(inst)
            new_function.blocks.append(copy.replace(block, instructions=out))
        new_module.functions.append(new_function)
    nc.m = new_module
    return nc


def _build_nc(C):
    import concourse.bass as bass
    import concourse.mybir as mybir
    from concourse.tile import TileContext

    fp32 = mybir.dt.float32
    bf16 = mybir.dt.bfloat16
    AF = mybir.ActivationFunctionType

    nch = -(-C // CC)
    chunks = [(i * CC, min(CC, C - i * CC)) for i in range(nch)]

    nc = bass.Bass()
    # All DRAM tensors are packed host-side so that one SBUF partition's data
    # is contiguous in DRAM (8-16KB per partition per transfer).
    xp = nc.declare_dram_parameter("xp", [nch, P, KO * CC], bf16, isOutput=False)
    w1p = nc.declare_dram_parameter("w1p", [FB, P, KO * FBLK], bf16, isOutput=False)
    w2p = nc.declare_dram_parameter("w2p", [G, P, GO * D], bf16, isOutput=False)
    b1 = nc.declare_dram_parameter("b1", [P, FT], fp32, isOutput=False)
    b2 = nc.declare_dram_parameter("b2", [P, DT], fp32, isOutput=False)
    yp = nc.declare_dram_parameter("yp", [nch, P, DT * CC], bf16, isOutput=True)

    with TileContext(nc) as tc:
        with (
            tc.tile_pool(name="wpool", bufs=1) as wpool,
            tc.tile_pool(name="xpool", bufs=2) as xpool,
            tc.tile_pool(name="hpool", bufs=1) as hpool,
            tc.tile_pool(name="ypool", bufs=2) as ypool,
            tc.tile_pool(name="hpsum", bufs=4, space="PSUM") as hpsum,
            tc.tile_pool(name="ypsum", bufs=4, space="PSUM") as ypsum,
        ):
            # DMA queue order == program order: chunk-0 activations + the
            # first W1 wave in half-transfers (the blockers for the first
            # real matmul), then the rest of the weights in deadline order.
            # The warm tile is memset on-device so the PE warmup loop can
            # start during the DMA ramp without waiting on any transfer.
            warm_sb = wpool.tile([P, P], bf16, tag="warm")
            nc.vector.memset(warm_sb[:], 0.0)

            # chunk-0 x and W1-wave-0 arrive in ko-pair quarters so the first
            # mm1 chain can start after only ~0.5MB has landed.
            x_sb = [None] * nch
            x_sb[0] = xpool.tile([P, KO * CC], bf16, tag="x", name="x0")
            w1_sb = [None] * FB
            w1_sb[0] = wpool.tile([P, KO * FBLK], bf16, tag="w1w0", name="w1s0")
            for j in range(4):
                nc.sync.dma_start(
                    x_sb[0][:, 2 * j * CC: 2 * (j + 1) * CC],
                    xp[0, :, 2 * j * CC: 2 * (j + 1) * CC])
                nc.sync.dma_start(
                    w1_sb[0][:, 2 * j * FBLK: 2 * (j + 1) * FBLK],
                    w1p[0, :, 2 * j * FBLK: 2 * (j + 1) * FBLK])
                if j == 0:
                    b1_sb = wpool.tile([P, FT], fp32, tag="b1")
                    nc.sync.dma_start(b1_sb[:], b1[:])

            b2_sb = wpool.tile([P, DT], fp32, tag="b2")
            nc.sync.dma_start(b2_sb[:], b2[:])

            for fb in range(1, FB):
                w1_sb[fb] = wpool.tile([P, KO * FBLK], bf16, tag=f"w1w{fb}",
                                       name=f"w1s{fb}")
                nc.sync.dma_start(w1_sb[fb][:], w1p[fb])
            w2_sb = [None] * G
            for g in range(G):
                w2_sb[g] = wpool.tile([P, GO * D], bf16, tag=f"w2g{g}",
                                      name=f"w2s{g}")
                nc.sync.dma_start(w2_sb[g][:], w2p[g])

            # PE warmup: full-size matmuls on the memset tile (no DMA deps)
            # keep the PE busy through the HAM activity window so the real
            # matmuls start at the full 2.4 GHz clock. Count is paced to end
            # roughly when the chunk-0 activations/weights land.
            warm_ps = hpsum.tile([P, CC], fp32, tag="hps")
            for _ in range(WARM_N):
                nc.tensor.matmul(
                    warm_ps[:, 0:64], warm_sb[:, 0:P], warm_sb[:, 0:64],
                    start=True, stop=True,
                )

            for ci, (c0, cn) in enumerate(chunks):
                if ci + 1 < nch:
                    x_sb[ci + 1] = xpool.tile([P, KO * CC], bf16, tag="x",
                                              name=f"x{ci + 1}")
                    nc.sync.dma_start(x_sb[ci + 1][:], xp[ci + 1])

                h_sb = hpool.tile([P, FT * CC], bf16, tag="h")
                for ft in range(FT):
                    fb, fc = divmod(ft * P, FBLK)
                    h_ps = hpsum.tile([P, CC], fp32, tag="hps")
                    for ko in range(KO):
                        nc.tensor.matmul(
                            h_ps[:, :cn],
                            w1_sb[fb][:, ko * FBLK + fc: ko * FBLK + fc + P],
                            x_sb[ci][:, ko * CC: ko * CC + cn],
                            start=(ko == 0),
                            stop=(ko == KO - 1),
                        )
                    # gelu(mm + b1) fused on ScalarE, cast to bf16 on write
                    nc.scalar.activation(
                        h_sb[:, ft * CC: ft * CC + cn], h_ps[:, :cn], AF.Gelu,
                        bias=b1_sb[:, ft:ft + 1],
                    )

                y_sb = ypool.tile([P, DT * CC], bf16, tag="y")
                for dt_ in range(DT):
                    y_ps = ypsum.tile([P, CC], fp32, tag="yps")
                    for fo in range(FT):
                        g, gl = divmod(fo, GO)
                        nc.tensor.matmul(
                            y_ps[:, :cn],
                            w2_sb[g][:, gl * D + dt_ * P: gl * D + dt_ * P + P],
                            h_sb[:, fo * CC: fo * CC + cn],
                            start=(fo == 0),
                            stop=(fo == FT - 1),
                        )
                    nc.vector.tensor_scalar_add(
                        y_sb[:, dt_ * CC: dt_ * CC + cn], y_ps[:, :cn],
                        b2_sb[:, dt_:dt_ + 1],
                    )
                if cn == CC:
                    nc.sync.dma_start(yp[ci], y_sb[:])
                else:
                    for dt_ in range(DT):
                        nc.sync.dma_start(
                            yp[ci, :, dt_ * CC: dt_ * CC + cn],
                            y_sb[:, dt_ * CC: dt_ * CC + cn],
                        )

    return _split_drain_waits(nc)


def _to_bf16(a):
    """Fast float32 -> bfloat16 with round-to-nearest-even via bit ops."""
    a = np.ascontiguousarray(a, dtype=np.float32)
    u = a.view(np.uint32)
    r = ((u + 0x7FFF + ((u >> 16) & 1)) >> 16).astype(np.uint16)
    return r.view(ml_dtypes.bfloat16)


def kernel(hidden_states, Wg, bg, W1, b1, W2, b2):
    from concourse import bass_utils

    hs = np.ascontiguousarray(hidden_states, dtype=np.float32).reshape(B * S, D)

    # ---- Gate on host (float64): softmax over experts, top-2, renormalize
    logits = hs.astype(np.float64) @ np.asarray(Wg, np.float64).T
    logits += np.asarray(bg, np.float64)
    logits -= logits.max(axis=-1, keepdims=True)
    p = np.exp(logits)
    p /= p.sum(axis=-1, keepdims=True)

    i1 = p.argmax(axis=-1)
    rows = np.arange(B * S)
    p1 = p[rows, i1]
    pm = p.copy()
    pm[rows, i1] = -1.0
    i2 = pm.argmax(axis=-1)
    p2 = p[rows, i2]
    denom = p1 + p2
    g1 = (p1 / denom).astype(np.float32)
    g2 = (p2 / denom).astype(np.float32)

    # ---- Dispatch: token ids + combine weights per expert
    ids, cws = [], []
    for e in range(E):
        m1 = np.nonzero(i1 == e)[0]
        m2 = np.nonzero(i2 == e)[0]
        ids.append(np.concatenate([m1, m2]))
        cws.append(np.concatenate([g1[m1], g2[m2]]))
    max_cnt = max(len(x) for x in ids)
    C = max(P, -(-max_cnt // P) * P)
    nch = -(-C // CC)

    if C not in _compiled:
        _compiled[C] = _build_nc(C)
    nc = _compiled[C]

    in_maps = []
    for e in range(E):
        xT = np.zeros((D, nch * CC), dtype=ml_dtypes.bfloat16)
        cnt = len(ids[e])
        xT[:, :cnt] = _to_bf16(hs[ids[e]]).T
        # pack: xp[ch, ki, ko*CC + c'] = xT[ko*P + ki, ch*CC + c']
        xpk = np.ascontiguousarray(
            xT.reshape(KO, P, nch, CC).transpose(2, 1, 0, 3).reshape(nch, P, KO * CC))
        # pack: w1p[fb, ki, ko*FBLK + f'] = W1[ko*P + ki, fb*FBLK + f']
        w1pk = np.ascontiguousarray(
            _to_bf16(W1[e]).reshape(KO, P, FB, FBLK)
            .transpose(2, 1, 0, 3).reshape(FB, P, KO * FBLK))
        # pack: w2p[g, fi, gl*D + d] = W2[(g*GO + gl)*P + fi, d]
        w2pk = np.ascontiguousarray(
            _to_bf16(W2[e]).reshape(G, GO, P, D)
            .transpose(0, 2, 1, 3).reshape(G, P, GO * D))
        in_maps.append({
            "xp": xpk,
            "w1p": w1pk,
            "w2p": w2pk,
            "b1": np.ascontiguousarray(
                np.asarray(b1[e], np.float32).reshape(FT, P).T),
            "b2": np.ascontiguousarray(
                np.asarray(b2[e], np.float32).reshape(DT, P).T),
        })

    kwargs = {}
    if TRACE:
        import os as _os
        kwargs = dict(trace=True, trace_cores=list(range(E)))
        if _os.environ.get("MOE_TRACE_DIR"):
            _os.makedirs(_os.environ["MOE_TRACE_DIR"], exist_ok=True)
            kwargs["tmpdir"] = _os.environ["MOE_TRACE_DIR"]
    res = bass_utils.run_bass_kernel_spmd(nc, in_maps, list(range(E)), **kwargs)
    global LAST_RESULTS
    LAST_RESULTS = res

    out = np.zeros((B * S, D), dtype=np.float32)
    for e in range(E):
        cnt = len(ids[e])
        # unpack: yp[ch, p, dt*CC + c'] = y[dt*P + p, ch*CC + c']
        ypk = np.asarray(res.results[e]["yp"], dtype=np.float32)
        yT = ypk.reshape(nch, P, DT, CC).transpose(2, 1, 0, 3).reshape(D, nch * CC)
        out[ids[e]] += cws[e][:, None] * yT[:, :cnt].T
    return out.reshape(B, S, D)
